# revision 21
# baseline (speedup 1.0000x reference)
"""Two-layer GAT (nn_GAT_82334523064895) on 8 TRN2 NeuronCores via Bass.

Strategy (8-way contiguous node sharding, SPMD single NEFF):
  1. h_aug = x_shard @ [W1 | W1@a_s1 | W1@a_d1] in bf16; x is pre-transposed
     on host (k-major), loaded in 8 large DMAs; PE accumulates h.T in PSUM
     over 64 k-chunks.
  2. h rows packed to 36B (16 x fp16 h + 1 x f32 s) in a dense slab;
     AllGather writes them strided into the 256B-row padded table (the
     dma_gather source granularity is 256B, but the collective only moves
     the useful bytes).
  3. Edge phase as padded ELL (no scatter): per 128-dst tile one dma_gather
     fetches table rows for all slots (slot 0 = self loop; pad slots hit a
     sentinel row whose s-value = -3e4 so exp -> exactly 0). Softmax without
     max-shift (edge logits are O(1)), weighted sums on DVE in fp16.
     d[dst] comes from the locally kept hr tiles, not the table.
  4. Layer-2 (C=2) via DVE matvec; 12B rows (h2[2], s2) AllGathered strided
     into table 2; same edge phase with d2 from local tiles.
  5. Global min/max via AllGather of per-core [max, -min] (cheaper than
     AllReduce) + local reduce; rescale on device.
Host does only integer index prep, sharding, transpose, dtype casts, and
unpermute.
"""

import numpy as np
import ml_dtypes

N = 8192
F = 8192
H = 16
C = 2
NCORES = 8
NSH = N // NCORES          # nodes per core
P = 128
NT = NSH // P              # dst tiles per core
AUG1 = H + 2               # h .. s, d
KCH = F // P               # k chunks
KGRP = 8                   # k chunks per x-load DMA
ROW = 64                   # f32 elements per padded table row (256B)
ROWH = 2 * ROW             # fp16 elements per padded table row
SL1 = 18                   # fp16 elems per dense slab-1 row (16 h + s + d)
SL2 = 3                    # f32 elems per dense slab-2 row (h2[2], s2)
NSHE = NSH + 1             # slab rows: NSH nodes + 1 sentinel row
PAD = N                    # sentinel marker in raw (node-id) index space
PADPOS = NSH               # sentinel position within core-0 block of the table
TROWS = NCORES * NSHE
NEG = 0.2
SENT = -30000.0
# gather pair schedule: big tile paired with small so the shared G buffer
# stays small; host emits the idx stream in this tile order
PAIRS = [(0, 7), (1, 6), (2, 5), (3, 4)]
TILE_ORDER = [t for pr in PAIRS for t in pr]


def _install_tilefix():
    """Split the Tile kernel-tail drain's sem waits across multiple drains
    (this walrus build rejects >1 sync wait on a CTRL instruction)."""
    import bass_rust
    from bass_rust import ScopedClock
    import concourse.tile as tile

    def _split_drain_and_barrier(self, tick_clock, wait_clock):
        nc = self.nc
        drain_inst = nc.sync.drain()
        wait_clock.add_sem_waits(
            drain_inst.ins, ScopedClock({None: tick_clock.global_clock})
        )
        si = drain_inst.ins.sync_info
        waits = list(si.on_wait) if si is not None else []
        if len(waits) > 1:
            si.on_wait = waits[:1]
            for i in range(1, len(waits)):
                d2 = nc.sync.drain()
                si2 = d2.ins.sync_info
                if si2 is None:
                    d2.ins.sync_info = bass_rust.SyncInfo(on_wait=[], on_update=[])
                    si2 = d2.ins.sync_info
                si2.on_wait = waits[i : i + 1]
        nc.all_engine_barrier()
        popped = nc._tile_sem_poison_stack.pop()
        assert popped is self._sem_poison
        nc.clear_and_free_semaphores(list(self.sems.allocated().values()))
        nc.all_engine_barrier()

    tile.TileContext._drain_and_barrier = _split_drain_and_barrier


def _split_multiwaits(d):
    """Walrus in this build accepts a single sync wait per instruction; hoist
    extra waits onto wait-only EventSemaphore carriers inserted just before."""
    n = 0
    for fn in d["functions"]:
        for blk in fn["blocks"]:
            newl = []
            for ins in blk["instructions"]:
                si = ins.get("sync_info")
                waits = (si or {}).get("on_wait") or []
                if len(waits) > 1:
                    for w in waits[:-1]:
                        n += 1
                        newl.append(
                            {
                                "debug": ins.get("debug"),
                                "engine": ins["engine"],
                                "ins": [],
                                "outs": [],
                                "name": f"{ins['name']}-ws{n}",
                                "opcode": "EventSemaphore",
                                "sync_info": {"on_update": [], "on_wait": [w]},
                            }
                        )
                    si["on_wait"] = [waits[-1]]
                newl.append(ins)
            blk["instructions"] = newl
    return d


def _patch_serialization(nc):
    import types
    import json

    orig = nc.to_json_bytes

    def to_json_bytes_patched(self):
        d = json.loads(orig())
        _split_multiwaits(d)
        return json.dumps(d).encode()

    nc.to_json_bytes = types.MethodType(to_json_bytes_patched, nc)


def _build(wts, phase="full"):
    import concourse.bass as bass
    import concourse.bacc as bacc
    import concourse.mybir as mybir
    import concourse.tile as tile
    from concourse.masks import make_identity
    import bass_rust

    _install_tilefix()
    dt = mybir.dt
    Alu = mybir.AluOpType
    Act = mybir.ActivationFunctionType
    RG = [list(range(NCORES))]

    wts = list(wts)
    IW = 8 * sum(wts)

    nc = bacc.Bacc("TRN2", debug=False)
    xs_p = nc.declare_dram_parameter("xs", [F, NSH], dt.bfloat16, isOutput=False)
    w1_p = nc.declare_dram_parameter("w1s", [P, KCH * AUG1], dt.bfloat16, isOutput=False)
    v1_p = nc.declare_dram_parameter("v1", [AUG1, 1], dt.float32, isOutput=False)
    idx1_p = nc.declare_dram_parameter("idx1", [P, IW], dt.int16, isOutput=False)
    idx2_p = nc.declare_dram_parameter("idx2", [P, IW], dt.int16, isOutput=False)
    w2b_p = nc.declare_dram_parameter("w2b", [P, 4 * H], dt.float32, isOutput=False)
    b2a_p = nc.declare_dram_parameter("b2a", [P, 4], dt.float32, isOutput=False)
    out_p = nc.declare_dram_parameter("out", [NSH, C], dt.float32, isOutput=True)

    with tile.TileContext(nc) as tc:
        with (
            tc.tile_pool(name="const", bufs=1) as cpool,
            tc.tile_pool(name="xload", bufs=1) as xpool,
            tc.tile_pool(name="work", bufs=3) as wpool,
            tc.tile_pool(name="gath", bufs=2) as gpool,
            tc.tile_pool(name="pst", bufs=2, space="PSUM") as ppool,
            tc.tile_pool(name="psacc", bufs=1, space="PSUM") as psacc,
            tc.tile_pool(name="dram", bufs=1, space="DRAM") as dpool,
        ):
            def _emit():
                # ---- weights needed by phase A first, so the x-load DMAs
                # start as early as possible
                w1_s = cpool.tile([P, KCH, AUG1], dt.bfloat16)
                nc.scalar.dma_start(w1_s[:], w1_p[:].rearrange("p (c f) -> p c f", f=AUG1))
                v1_s = cpool.tile([AUG1, 1], dt.float32)
                nc.scalar.dma_start(v1_s[:], v1_p[:])

                # ---- internal DRAM
                NH2 = NSH // 2
                l1slabA = dpool.tile([NH2, SL1], dt.float16)
                l1slabB = dpool.tile([NH2 + 1, SL1], dt.float16)
                t1dA = dpool.tile([NCORES * NH2, SL1], dt.float16, addr_space="Shared")
                t1dB = dpool.tile(
                    [NCORES * (NH2 + 1), SL1], dt.float16, addr_space="Shared"
                )
                table1 = dpool.tile([TROWS, ROW], dt.float32)
                l2slab = dpool.tile([NSHE, SL2], dt.float16)
                t2d = dpool.tile([TROWS, SL2], dt.float16, addr_space="Shared")
                table2 = dpool.tile([TROWS, ROW], dt.float32)
                mmx_i = dpool.tile([1, 2], dt.float32)
                mmx_o = dpool.tile([1, 2 * NCORES], dt.float32, addr_space="Shared")


                # ---- Phase A: hT = W1aug.T @ x.T accumulated over k chunks
                # x arrives host-pre-transposed [F, NSH]; load in KGRP-chunk
                # groups so PE pipelines behind the DMA stream.
                hps0 = psacc.tile([AUG1, 512], dt.float32, tag="hps0")
                hps1 = psacc.tile([AUG1, 512], dt.float32, tag="hps1")
                hps = [hps0, hps1]
                xts = {}
                for half in range(2):
                    for g in range(KCH // KGRP):
                        xt = xpool.tile(
                            [P, KGRP, NSH // 2], dt.bfloat16, tag=f"xt{half}_{g}"
                        )
                        nc.sync.dma_start(
                            xt[:],
                            xs_p[
                                g * KGRP * P : (g + 1) * KGRP * P,
                                half * NH2 : (half + 1) * NH2,
                            ].rearrange("(c p) n -> p c n", p=P),
                        )
                        xts[(half, g)] = xt
                # remaining constants (needed only from the edge phase on)
                idx1_s = cpool.tile([P, IW], dt.int16)
                nc.sync.dma_start(idx1_s[:], idx1_p[:])
                idx2_s = cpool.tile([P, IW], dt.int16)
                nc.sync.dma_start(idx2_s[:], idx2_p[:])
                w2b_s = cpool.tile([P, 4, H], dt.float32)
                nc.sync.dma_start(w2b_s[:], w2b_p[:].rearrange("p (c k) -> p c k", k=H))
                b2a_s = cpool.tile([P, 4], dt.float32)
                nc.sync.dma_start(b2a_s[:], b2a_p[:])
                ident = cpool.tile([P, P], dt.float32)
                make_identity(nc, ident[:])
                # sentinel rows ride along in each core's slab (Shared tables
                # may only be written by the collective itself)
                sent1 = cpool.tile([1, SL1], dt.float16)
                nc.gpsimd.memset(sent1[:], 0.0)
                nc.gpsimd.memset(sent1[:, H : H + 1], SENT)
                nc.scalar.dma_start(l1slabB[NH2 : NH2 + 1, :], sent1[:])
                sent2 = cpool.tile([1, SL2], dt.float16)
                nc.gpsimd.memset(sent2[:], 0.0)
                nc.gpsimd.memset(sent2[:, SL2 - 1 :], SENT)
                nc.scalar.dma_start(l2slab[NSH : NSH + 1, :], sent2[:])
                # ---- Phase A+B fused per half: matmuls, then pack + slab
                # + (for half 0) the first AllGather, so AG1a overlaps the
                # second half of the x stream
                hT = cpool.tile([AUG1, NSH], dt.float32)
                NT2 = NT // 2
                for half in range(2):
                    for g in range(KCH // KGRP):
                        xt = xts[(half, g)]
                        for j in range(KGRP):
                            ck = g * KGRP + j
                            nc.tensor.matmul(
                                hps[half][:],
                                w1_s[:, ck, :],
                                xt[:, j, :],
                                start=(ck == 0),
                                stop=(ck == KCH - 1),
                            )
                    nc.scalar.activation(
                        hT[:, half * NH2 : (half + 1) * NH2],
                        hps[half][:],
                        Act.Identity,
                        bias=v1_s[:],
                    )
                    rows = cpool.tile([P, NT2, SL1], dt.float16, tag=f"rows{half}")
                    for tt in range(NT2):
                        t = half * NT2 + tt
                        hr_ps = ppool.tile([P, AUG1], dt.float32, tag="hrps")
                        nc.tensor.transpose(
                            hr_ps[:], hT[:, t * P : (t + 1) * P], ident[:AUG1, :AUG1]
                        )
                        hr = wpool.tile([P, AUG1], dt.float32, tag="hr")
                        nc.vector.tensor_copy(hr[:], hr_ps[:])
                        nc.vector.tensor_copy(rows[:, tt, :], hr[:])
                    slab = l1slabA if half == 0 else l1slabB
                    nc.scalar.dma_start(
                        slab[0:NH2, :].rearrange("(t p) c -> p t c", p=P), rows[:]
                    )
                    src_d, dst_d, r0 = (
                        (l1slabA, t1dA, 0)
                        if half == 0
                        else (l1slabB, t1dB, NCORES * NH2)
                    )
                    nc.gpsimd.collective_compute(
                        "AllGather",
                        Alu.bypass,
                        replica_groups=RG,
                        ins=[src_d[:].opt()],
                        outs=[dst_d[:].opt()],
                    )
                    nc.scalar.dma_start(
                        table1[:].bitcast(dt.float16)[r0 : r0 + dst_d.shape[0], 0:SL1],
                        dst_d[:],
                    )

                allout = cpool.tile([P, NT, C], dt.float32)
                h2bs = {}

                rows2 = cpool.tile([P, NT, SL2], dt.float16)

                def edge_layer1(table, idx_s):
                    off = 0
                    for ta, tb in PAIRS:
                        Wp = wts[ta] + wts[tb]
                        ni = P * Wp
                        Gp = gpool.tile([P, Wp, ROW], dt.float32, tag="G")
                        nc.gpsimd.dma_gather(
                            out_ap=Gp[:],
                            in_ap=table[:],
                            idxs_ap=idx_s[:, off : off + 8 * Wp],
                            num_idxs=ni,
                            num_idxs_reg=ni,
                            elem_size=ROW,
                            single_packet=True,
                        )
                        off += 8 * Wp
                        for t, s0 in ((ta, 0), (tb, wts[ta])):
                            Wt = wts[t]
                            G = Gp[:, s0 : s0 + Wt, :]
                            # z = s[src] + d[dst(self)]
                            z = wpool.tile([P, Wt], dt.float32, tag="z")
                            sdv = G[:, 0:Wt, H // 2 : H // 2 + 1].bitcast(dt.float16)
                            nc.scalar.activation(
                                z[:],
                                sdv[:, :, 0:1].squeeze(),
                                Act.Identity,
                                bias=G[:, 0:1, H // 2 : H // 2 + 1]
                                .bitcast(dt.float16)[:, :, 1:2]
                                .rearrange("p a b -> p (a b)"),
                            )
                            # e = max(z, 0.2 z)  (leaky relu)
                            e = wpool.tile([P, Wt], dt.float32, tag="e")
                            nc.vector.scalar_tensor_tensor(
                                out=e[:], in0=z[:], scalar=NEG, in1=z[:],
                                op0=Alu.mult, op1=Alu.max,
                            )
                            # ex = exp(e), den = sum(ex)
                            ex = wpool.tile([P, Wt], dt.float16, tag="ex")
                            den = wpool.tile([P, 1], dt.float32, tag="den")
                            nc.scalar.activation(ex[:], e[:], Act.Exp, accum_out=den[:])
                            rec = wpool.tile([P, 1], dt.float32, tag="rec")
                            nc.vector.reciprocal(rec[:], den[:])
                            # num[p,f] = sum_s ex[p,s] * h16[p,s,f]
                            tmp = wpool.tile([P, H, Wt], dt.float32, tag="tmp1")
                            nc.vector.tensor_tensor(
                                out=tmp[:],
                                in0=G[:, 0:Wt, 0 : H // 2]
                                .bitcast(dt.float16)
                                .rearrange("p s f -> p f s"),
                                in1=ex[:].unsqueeze(1).to_broadcast([P, H, Wt]),
                                op=Alu.mult,
                            )
                            num = wpool.tile([P, H], dt.float32, tag="num1")
                            nc.vector.tensor_reduce(
                                num[:], tmp[:], axis=mybir.AxisListType.X, op=Alu.add
                            )
                            o1 = wpool.tile([P, H], dt.float32, tag="o1")
                            nc.vector.tensor_scalar_mul(o1[:], num[:], rec[:])
                            # h2_aug = o1 @ W2aug (+b2 pattern) on DVE
                            tmp2 = wpool.tile([P, 4, H], dt.float32, tag="tmp2")
                            nc.vector.tensor_tensor(
                                out=tmp2[:],
                                in0=o1[:].unsqueeze(1).to_broadcast([P, 4, H]),
                                in1=w2b_s[:],
                                op=Alu.mult,
                            )
                            h2t = wpool.tile([P, 4], dt.float32, tag="h2t")
                            nc.vector.tensor_reduce(
                                h2t[:], tmp2[:], axis=mybir.AxisListType.X, op=Alu.add
                            )
                            h2b = cpool.tile([P, 4], dt.float32, tag=f"h2b{t}")
                            nc.vector.tensor_add(h2b[:], h2t[:], b2a_s[:])
                            h2bs[t] = h2b
                            nc.vector.tensor_copy(rows2[:, t, :], h2b[:, 0:SL2])

                def edge_layer2(table, idx_s):
                    off = 0
                    for ta, tb in PAIRS:
                        Wp = wts[ta] + wts[tb]
                        ni = P * Wp
                        Gp = gpool.tile([P, Wp, ROW], dt.float32, tag="G")
                        nc.gpsimd.dma_gather(
                            out_ap=Gp[:],
                            in_ap=table[:],
                            idxs_ap=idx_s[:, off : off + 8 * Wp],
                            num_idxs=ni,
                            num_idxs_reg=ni,
                            elem_size=ROW,
                            single_packet=True,
                        )
                        off += 8 * Wp
                        for t, s0 in ((ta, 0), (tb, wts[ta])):
                            Wt = wts[t]
                            G = Gp[:, s0 : s0 + Wt, :]
                            z = wpool.tile([P, Wt], dt.float32, tag="z")
                            nc.scalar.activation(
                                z[:],
                                G[:, 0:Wt, 1:2]
                                .bitcast(dt.float16)[:, :, 0:1]
                                .squeeze(),
                                Act.Identity,
                                bias=h2bs[t][:, 3:4],
                            )
                            e = wpool.tile([P, Wt], dt.float32, tag="e")
                            nc.vector.scalar_tensor_tensor(
                                out=e[:], in0=z[:], scalar=NEG, in1=z[:],
                                op0=Alu.mult, op1=Alu.max,
                            )
                            ex = wpool.tile([P, Wt], dt.float16, tag="ex2")
                            den = wpool.tile([P, 1], dt.float32, tag="den")
                            nc.scalar.activation(ex[:], e[:], Act.Exp, accum_out=den[:])
                            rec = wpool.tile([P, 1], dt.float32, tag="rec")
                            nc.vector.reciprocal(rec[:], den[:])
                            tmp = wpool.tile([P, C, Wt], dt.float32, tag="tmp2c")
                            nc.vector.tensor_tensor(
                                out=tmp[:],
                                in0=G[:, 0:Wt, 0:1]
                                .bitcast(dt.float16)
                                .rearrange("p s f -> p f s"),
                                in1=ex[:].unsqueeze(1).to_broadcast([P, C, Wt]),
                                op=Alu.mult,
                            )
                            num = wpool.tile([P, C], dt.float32, tag="num2")
                            nc.vector.tensor_reduce(
                                num[:], tmp[:], axis=mybir.AxisListType.X, op=Alu.add
                            )
                            nc.vector.tensor_scalar_mul(allout[:, t, :], num[:], rec[:])

                # ---- Phase C: layer-1 edge aggregation + h2
                if phase == "gemm":
                    nc.sync.dma_start(out_p[0:NH2, :], l1slabA[0:NH2, 0:C])
                    return
                if phase == "ag1":
                    nc.sync.dma_start(out_p[:], table1[0:NSH, 0:C])
                    return
                edge_layer1(table1, idx1_s)
                nc.scalar.dma_start(
                    l2slab[0:NSH, :].rearrange("(t p) c -> p t c", p=P), rows2[:]
                )
                if phase == "gat1":
                    nc.sync.dma_start(out_p[:], l2slab[0:NSH, 0:C])
                    return
                nc.gpsimd.collective_compute(
                    "AllGather",
                    Alu.bypass,
                    replica_groups=RG,
                    ins=[l2slab[:].opt()],
                    outs=[t2d[:].opt()],
                )
                nc.scalar.dma_start(
                    table2[:].bitcast(dt.float16)[0:TROWS, 0:SL2], t2d[:]
                )
                if phase == "ag2":
                    nc.sync.dma_start(out_p[:], table2[0:NSH, 0:C])  # debug only
                    return
                # ---- Phase D: layer-2 edge aggregation
                edge_layer2(table2, idx2_s)
                if phase == "gat2":
                    for t in range(NT):
                        nc.sync.dma_start(out_p[t * P : (t + 1) * P, :], allout[:, t, :])
                    return

                # ---- Phase E: global min/max + rescale
                mm = wpool.tile([P, 2], dt.float32, tag="mm")
                nc.vector.tensor_reduce(
                    mm[:, 0:1], allout[:], axis=mybir.AxisListType.XY, op=Alu.max
                )
                mnt = wpool.tile([P, 1], dt.float32, tag="mnt")
                nc.vector.tensor_reduce(
                    mnt[:], allout[:], axis=mybir.AxisListType.XY, op=Alu.min
                )
                nc.vector.tensor_scalar_mul(mm[:, 1:2], mnt[:], -1.0)
                pr = wpool.tile([P, 2], dt.float32, tag="pr")
                nc.gpsimd.partition_all_reduce(
                    pr[:], mm[:], channels=P, reduce_op=bass_rust.ReduceOp.max
                )
                nc.sync.dma_start(mmx_i[:], pr[0:1, :])
                nc.gpsimd.collective_compute(
                    "AllGather",
                    Alu.bypass,
                    replica_groups=RG,
                    ins=[mmx_i[:].opt()],
                    outs=[mmx_o[:].opt()],
                )
                mmr = wpool.tile([1, NCORES * 2], dt.float32, tag="mmr")
                nc.sync.dma_start(mmr[:], mmx_o[:])
                gmx = wpool.tile([1, 2], dt.float32, tag="gmx")
                nc.vector.tensor_reduce(
                    gmx[:],
                    mmr[:].rearrange("p (r c) -> p c r", c=2),
                    axis=mybir.AxisListType.X,
                    op=Alu.max,
                )
                bc = wpool.tile([P, 2], dt.float32, tag="bc")
                nc.gpsimd.partition_broadcast(bc[:], gmx[:])
                # scale = 2/(mx-mn); shift = 2*(-mn)/(mx-mn) - 1
                rng_ = wpool.tile([P, 1], dt.float32, tag="rng")
                nc.vector.tensor_tensor(rng_[:], bc[:, 0:1], bc[:, 1:2], op=Alu.add)
                ri = wpool.tile([P, 1], dt.float32, tag="ri")
                nc.vector.reciprocal(ri[:], rng_[:])
                sc = wpool.tile([P, 1], dt.float32, tag="sc")
                nc.vector.tensor_scalar_mul(sc[:], ri[:], 2.0)
                u = wpool.tile([P, 1], dt.float32, tag="u")
                nc.vector.tensor_tensor(u[:], bc[:, 1:2], ri[:], op=Alu.mult)
                sh = wpool.tile([P, 1], dt.float32, tag="sh")
                nc.vector.tensor_scalar(
                    out=sh[:], in0=u[:], scalar1=2.0, scalar2=-1.0,
                    op0=Alu.mult, op1=Alu.add,
                )
                fin = wpool.tile([P, NT, C], dt.float32, tag="fin")
                nc.vector.tensor_scalar(
                    out=fin[:], in0=allout[:], scalar1=sc[:], scalar2=sh[:],
                    op0=Alu.mult, op1=Alu.add,
                )
                nc.sync.dma_start(
                    out_p[:].rearrange("(t p) c -> p t c", p=P), fin[:]
                )

            _emit()
    nc.compile()
    _patch_serialization(nc)
    return nc


def _prep(x, edge_index, W1, a_src1, a_dst1, b1, W2, a_src2, a_dst2, b2):
    ei = np.asarray(edge_index).astype(np.int64)
    src_all, dst_all = ei[0], ei[1]
    counts = np.bincount(dst_all, minlength=N)
    perm_e = np.argsort(dst_all, kind="stable")
    ssorted = src_all[perm_e].astype(np.int64)
    starts = np.zeros(N + 1, np.int64)
    np.cumsum(counts, out=starts[1:])

    orders = []
    wt_core = np.zeros((NCORES, NT), np.int64)
    for c in range(NCORES):
        ids = np.arange(NSH * c, NSH * (c + 1))
        o = ids[np.argsort(-counts[ids], kind="stable")]
        orders.append(o)
        for t in range(NT):
            wt_core[c, t] = 1 + counts[o[P * t]]
    wts = tuple(int(w) for w in wt_core.max(axis=0))

    # table-1 positions (two AllGather regions: first node-halves of every
    # core, then second halves incl. per-core sentinel rows)
    NH2 = NSH // 2
    pos1 = np.empty(N + 1, np.int64)
    g = np.arange(N)
    c, o = g // NSH, g % NSH
    pos1[g] = np.where(
        o < NH2, c * NH2 + o, NCORES * NH2 + c * (NH2 + 1) + (o - NH2)
    )
    pos1[PAD] = NCORES * NH2 + NH2
    pos2 = np.empty(N + 1, np.int64)
    pos2[PAD] = PADPOS
    for c in range(NCORES):
        pos2[orders[c]] = NSHE * c + np.arange(NSH)

    idx1_maps, idx2_maps = [], []
    for c in range(NCORES):
        segs1 = []
        for t in range(NT):
            wt = wts[t]
            nodes = orders[c][P * t : P * (t + 1)]
            mat = np.full((wt, P), PAD, np.int64)
            mat[0, :] = nodes
            for p, g in enumerate(nodes):
                dg = counts[g]
                mat[1 : 1 + dg, p] = ssorted[starts[g] : starts[g] + dg]
            segs1.append(mat)
        raw = np.concatenate([m.reshape(-1) for m in segs1])
        unwrap1 = pos1[raw]
        unwrap2 = pos2[raw]

        def wrap(unwrap):
            starts_t = np.zeros(NT + 1, np.int64)
            np.cumsum([P * w for w in wts], out=starts_t[1:])
            parts = []
            for t in TILE_ORDER:
                o = starts_t[t]
                parts.append(unwrap[o : o + P * wts[t]].reshape(-1, 16).T)
            w16 = np.concatenate(parts, axis=1).astype(np.int16)
            return np.tile(w16, (NCORES, 1))

        idx1_maps.append(wrap(unwrap1))
        idx2_maps.append(wrap(unwrap2))

    bf = ml_dtypes.bfloat16
    W1aug = np.concatenate(
        [W1, (W1 @ a_src1)[:, None], (W1 @ a_dst1)[:, None]], axis=1
    ).astype(np.float32)
    w1s = (
        W1aug.reshape(KCH, P, AUG1).transpose(1, 0, 2).reshape(P, KCH * AUG1)
    ).astype(bf)
    v1 = np.concatenate([b1.astype(np.float32), np.zeros(2, np.float32)]).reshape(
        AUG1, 1
    )
    W2aug = np.concatenate(
        [W2, (W2 @ a_src2)[:, None], (W2 @ a_dst2)[:, None]], axis=1
    ).astype(np.float32)
    w2b = np.tile(W2aug.T.reshape(1, 4 * H), (P, 1)).astype(np.float32)
    b2a = np.tile(
        np.array([b2[0], b2[1], 0.0, 0.0], np.float32), (P, 1)
    ).astype(np.float32)

    x = np.asarray(x, np.float32)
    in_maps = []
    for c in range(NCORES):
        in_maps.append(
            {
                "xs": np.ascontiguousarray(x[NSH * c : NSH * (c + 1)].T).astype(bf),
                "w1s": w1s,
                "v1": v1,
                "idx1": idx1_maps[c],
                "idx2": idx2_maps[c],
                "w2b": w2b,
                "b2a": b2a,
            }
        )
    return wts, in_maps, orders


_NC_CACHE = {}


def _get_nc(wts):
    if wts not in _NC_CACHE:
        _NC_CACHE[wts] = _build(wts)
    return _NC_CACHE[wts]


def kernel(**inputs):
    from concourse.bass_utils import run_bass_kernel_spmd

    wts, in_maps, orders = _prep(
        inputs["x"], inputs["edge_index"], inputs["W1"], inputs["a_src1"],
        inputs["a_dst1"], inputs["b1"], inputs["W2"], inputs["a_src2"],
        inputs["a_dst2"], inputs["b2"],
    )
    nc = _get_nc(wts)
    res = run_bass_kernel_spmd(nc, in_maps, list(range(NCORES)))
    out = np.empty((N, C), np.float32)
    for c in range(NCORES):
        out[orders[c]] = res.results[c]["out"]
    return out


# revision 22
# speedup vs baseline: 1.0071x; 1.0071x over previous
"""Two-layer GAT (nn_GAT_82334523064895) on 8 TRN2 NeuronCores via Bass.

Strategy (8-way contiguous node sharding, SPMD single NEFF):
  1. h_aug = x_shard @ [W1 | W1@a_s1 | W1@a_d1] in bf16; x is pre-transposed
     on host (k-major), loaded in 8 large DMAs; PE accumulates h.T in PSUM
     over 64 k-chunks.
  2. h rows packed to 36B (16 x fp16 h + 1 x f32 s) in a dense slab;
     AllGather writes them strided into the 256B-row padded table (the
     dma_gather source granularity is 256B, but the collective only moves
     the useful bytes).
  3. Edge phase as padded ELL (no scatter): per 128-dst tile one dma_gather
     fetches table rows for all slots (slot 0 = self loop; pad slots hit a
     sentinel row whose s-value = -3e4 so exp -> exactly 0). Softmax without
     max-shift (edge logits are O(1)), weighted sums on DVE in fp16.
     d[dst] comes from the locally kept hr tiles, not the table.
  4. Layer-2 (C=2) via DVE matvec; 12B rows (h2[2], s2) AllGathered strided
     into table 2; same edge phase with d2 from local tiles.
  5. Global min/max via AllGather of per-core [max, -min] (cheaper than
     AllReduce) + local reduce; rescale on device.
Host does only integer index prep, sharding, transpose, dtype casts, and
unpermute.
"""

import numpy as np
import ml_dtypes

N = 8192
F = 8192
H = 16
C = 2
NCORES = 8
NSH = N // NCORES          # nodes per core
P = 128
NT = NSH // P              # dst tiles per core
AUG1 = H + 2               # h .. s, d
KCH = F // P               # k chunks
KGRP = 8                   # k chunks per x-load DMA
ROW = 64                   # f32 elements per padded table row (256B)
ROWH = 2 * ROW             # fp16 elements per padded table row
SL1 = 18                   # fp16 elems per dense slab-1 row (16 h + s + d)
SL2 = 3                    # f32 elems per dense slab-2 row (h2[2], s2)
NSHE = NSH + 1             # slab rows: NSH nodes + 1 sentinel row
PAD = N                    # sentinel marker in raw (node-id) index space
PADPOS = NSH               # sentinel position within core-0 block of the table
TROWS = NCORES * NSHE
NEG = 0.2
SENT = -30000.0
# gather pair schedule: big tile paired with small so the shared G buffer
# stays small; host emits the idx stream in this tile order
PAIRS = [(0, 7), (1, 6), (2, 5), (3, 4)]
GROUPS = [[t] for t in range(8)]
TILE_ORDER = [t for gr in GROUPS for t in gr]


def _install_tilefix():
    """Split the Tile kernel-tail drain's sem waits across multiple drains
    (this walrus build rejects >1 sync wait on a CTRL instruction)."""
    import bass_rust
    from bass_rust import ScopedClock
    import concourse.tile as tile

    def _split_drain_and_barrier(self, tick_clock, wait_clock):
        nc = self.nc
        drain_inst = nc.sync.drain()
        wait_clock.add_sem_waits(
            drain_inst.ins, ScopedClock({None: tick_clock.global_clock})
        )
        si = drain_inst.ins.sync_info
        waits = list(si.on_wait) if si is not None else []
        if len(waits) > 1:
            si.on_wait = waits[:1]
            for i in range(1, len(waits)):
                d2 = nc.sync.drain()
                si2 = d2.ins.sync_info
                if si2 is None:
                    d2.ins.sync_info = bass_rust.SyncInfo(on_wait=[], on_update=[])
                    si2 = d2.ins.sync_info
                si2.on_wait = waits[i : i + 1]
        nc.all_engine_barrier()
        popped = nc._tile_sem_poison_stack.pop()
        assert popped is self._sem_poison
        nc.clear_and_free_semaphores(list(self.sems.allocated().values()))
        nc.all_engine_barrier()

    tile.TileContext._drain_and_barrier = _split_drain_and_barrier


def _split_multiwaits(d):
    """Walrus in this build accepts a single sync wait per instruction; hoist
    extra waits onto wait-only EventSemaphore carriers inserted just before."""
    n = 0
    for fn in d["functions"]:
        for blk in fn["blocks"]:
            newl = []
            for ins in blk["instructions"]:
                si = ins.get("sync_info")
                waits = (si or {}).get("on_wait") or []
                if len(waits) > 1:
                    for w in waits[:-1]:
                        n += 1
                        newl.append(
                            {
                                "debug": ins.get("debug"),
                                "engine": ins["engine"],
                                "ins": [],
                                "outs": [],
                                "name": f"{ins['name']}-ws{n}",
                                "opcode": "EventSemaphore",
                                "sync_info": {"on_update": [], "on_wait": [w]},
                            }
                        )
                    si["on_wait"] = [waits[-1]]
                newl.append(ins)
            blk["instructions"] = newl
    return d


def _patch_serialization(nc):
    import types
    import json

    orig = nc.to_json_bytes

    def to_json_bytes_patched(self):
        d = json.loads(orig())
        _split_multiwaits(d)
        return json.dumps(d).encode()

    nc.to_json_bytes = types.MethodType(to_json_bytes_patched, nc)


def _build(wts, phase="full"):
    import concourse.bass as bass
    import concourse.bacc as bacc
    import concourse.mybir as mybir
    import concourse.tile as tile
    from concourse.masks import make_identity
    import bass_rust

    _install_tilefix()
    dt = mybir.dt
    Alu = mybir.AluOpType
    Act = mybir.ActivationFunctionType
    RG = [list(range(NCORES))]

    wts = list(wts)
    IW = 8 * sum(wts)

    nc = bacc.Bacc("TRN2", debug=False)
    xs_p = nc.declare_dram_parameter("xs", [F, NSH], dt.bfloat16, isOutput=False)
    w1_p = nc.declare_dram_parameter("w1s", [P, KCH * AUG1], dt.bfloat16, isOutput=False)
    v1_p = nc.declare_dram_parameter("v1", [AUG1, 1], dt.float32, isOutput=False)
    idx1_p = nc.declare_dram_parameter("idx1", [P, IW], dt.int16, isOutput=False)
    idx2_p = nc.declare_dram_parameter("idx2", [P, IW], dt.int16, isOutput=False)
    w2b_p = nc.declare_dram_parameter("w2b", [P, 4 * H], dt.float32, isOutput=False)
    b2a_p = nc.declare_dram_parameter("b2a", [P, 4], dt.float32, isOutput=False)
    out_p = nc.declare_dram_parameter("out", [NSH, C], dt.float32, isOutput=True)

    with tile.TileContext(nc) as tc:
        with (
            tc.tile_pool(name="const", bufs=1) as cpool,
            tc.tile_pool(name="xload", bufs=1) as xpool,
            tc.tile_pool(name="work", bufs=3) as wpool,
            tc.tile_pool(name="gath", bufs=2) as gpool,
            tc.tile_pool(name="pst", bufs=2, space="PSUM") as ppool,
            tc.tile_pool(name="psacc", bufs=1, space="PSUM") as psacc,
            tc.tile_pool(name="dram", bufs=1, space="DRAM") as dpool,
        ):
            def _emit():
                # ---- weights needed by phase A first, so the x-load DMAs
                # start as early as possible
                w1_s = cpool.tile([P, KCH, AUG1], dt.bfloat16)
                nc.scalar.dma_start(w1_s[:], w1_p[:].rearrange("p (c f) -> p c f", f=AUG1))
                v1_s = cpool.tile([AUG1, 1], dt.float32)
                nc.scalar.dma_start(v1_s[:], v1_p[:])

                # ---- internal DRAM
                NH2 = NSH // 2
                l1slabA = dpool.tile([NH2, SL1], dt.float16)
                l1slabB = dpool.tile([NH2 + 1, SL1], dt.float16)
                t1dA = dpool.tile([NCORES * NH2, SL1], dt.float16, addr_space="Shared")
                t1dB = dpool.tile(
                    [NCORES * (NH2 + 1), SL1], dt.float16, addr_space="Shared"
                )
                table1 = dpool.tile([TROWS, ROW], dt.float32)
                l2slab = dpool.tile([NSHE, SL2], dt.float16)
                t2d = dpool.tile([TROWS, SL2], dt.float16, addr_space="Shared")
                table2 = dpool.tile([TROWS, ROW], dt.float32)
                mmx_i = dpool.tile([1, 2], dt.float32)
                mmx_o = dpool.tile([1, 2 * NCORES], dt.float32, addr_space="Shared")


                # ---- Phase A: hT = W1aug.T @ x.T accumulated over k chunks
                # x arrives host-pre-transposed [F, NSH]; load in KGRP-chunk
                # groups so PE pipelines behind the DMA stream.
                hps0 = psacc.tile([AUG1, 512], dt.float32, tag="hps0")
                hps1 = psacc.tile([AUG1, 512], dt.float32, tag="hps1")
                hps = [hps0, hps1]
                xts = {}
                for half in range(2):
                    for g in range(KCH // KGRP):
                        xt = xpool.tile(
                            [P, KGRP, NSH // 2], dt.bfloat16, tag=f"xt{half}_{g}"
                        )
                        nc.sync.dma_start(
                            xt[:],
                            xs_p[
                                g * KGRP * P : (g + 1) * KGRP * P,
                                half * NH2 : (half + 1) * NH2,
                            ].rearrange("(c p) n -> p c n", p=P),
                        )
                        xts[(half, g)] = xt
                # remaining constants (needed only from the edge phase on)
                idx1_s = cpool.tile([P, IW], dt.int16)
                nc.sync.dma_start(idx1_s[:], idx1_p[:])
                idx2_s = cpool.tile([P, IW], dt.int16)
                nc.sync.dma_start(idx2_s[:], idx2_p[:])
                w2b_s = cpool.tile([P, 4, H], dt.float32)
                nc.sync.dma_start(w2b_s[:], w2b_p[:].rearrange("p (c k) -> p c k", k=H))
                b2a_s = cpool.tile([P, 4], dt.float32)
                nc.sync.dma_start(b2a_s[:], b2a_p[:])
                ident = cpool.tile([P, P], dt.float32)
                make_identity(nc, ident[:])
                # sentinel rows ride along in each core's slab (Shared tables
                # may only be written by the collective itself)
                sent1 = cpool.tile([1, SL1], dt.float16)
                nc.gpsimd.memset(sent1[:], 0.0)
                nc.gpsimd.memset(sent1[:, H : H + 1], SENT)
                nc.scalar.dma_start(l1slabB[NH2 : NH2 + 1, :], sent1[:])
                sent2 = cpool.tile([1, SL2], dt.float16)
                nc.gpsimd.memset(sent2[:], 0.0)
                nc.gpsimd.memset(sent2[:, SL2 - 1 :], SENT)
                nc.scalar.dma_start(l2slab[NSH : NSH + 1, :], sent2[:])
                # ---- Phase A+B fused per half: matmuls, then pack + slab
                # + (for half 0) the first AllGather, so AG1a overlaps the
                # second half of the x stream
                hT = cpool.tile([AUG1, NSH], dt.float32)
                NT2 = NT // 2
                for half in range(2):
                    for g in range(KCH // KGRP):
                        xt = xts[(half, g)]
                        for j in range(KGRP):
                            ck = g * KGRP + j
                            nc.tensor.matmul(
                                hps[half][:],
                                w1_s[:, ck, :],
                                xt[:, j, :],
                                start=(ck == 0),
                                stop=(ck == KCH - 1),
                            )
                    nc.scalar.activation(
                        hT[:, half * NH2 : (half + 1) * NH2],
                        hps[half][:],
                        Act.Identity,
                        bias=v1_s[:],
                    )
                    rows = cpool.tile([P, NT2, SL1], dt.float16, tag=f"rows{half}")
                    for tt in range(NT2):
                        t = half * NT2 + tt
                        hr_ps = ppool.tile([P, AUG1], dt.float32, tag="hrps")
                        nc.tensor.transpose(
                            hr_ps[:], hT[:, t * P : (t + 1) * P], ident[:AUG1, :AUG1]
                        )
                        hr = wpool.tile([P, AUG1], dt.float32, tag="hr")
                        nc.vector.tensor_copy(hr[:], hr_ps[:])
                        nc.vector.tensor_copy(rows[:, tt, :], hr[:])
                    slab = l1slabA if half == 0 else l1slabB
                    nc.scalar.dma_start(
                        slab[0:NH2, :].rearrange("(t p) c -> p t c", p=P), rows[:]
                    )
                    src_d, dst_d, r0 = (
                        (l1slabA, t1dA, 0)
                        if half == 0
                        else (l1slabB, t1dB, NCORES * NH2)
                    )
                    nc.gpsimd.collective_compute(
                        "AllGather",
                        Alu.bypass,
                        replica_groups=RG,
                        ins=[src_d[:].opt()],
                        outs=[dst_d[:].opt()],
                    )
                    nc.scalar.dma_start(
                        table1[:].bitcast(dt.float16)[r0 : r0 + dst_d.shape[0], 0:SL1],
                        dst_d[:],
                    )

                allout = cpool.tile([P, NT, C], dt.float32)
                h2bs = {}

                rows2 = cpool.tile([P, NT, SL2], dt.float16)

                def edge_layer1(table, idx_s):
                    off = 0
                    for grp in GROUPS:
                        Wp = sum(wts[t] for t in grp)
                        ni = P * Wp
                        Gp = gpool.tile([P, Wp, ROW], dt.float32, tag="G")
                        nc.gpsimd.dma_gather(
                            out_ap=Gp[:],
                            in_ap=table[:],
                            idxs_ap=idx_s[:, off : off + 8 * Wp],
                            num_idxs=ni,
                            num_idxs_reg=ni,
                            elem_size=ROW,
                            single_packet=False,
                        )
                        off += 8 * Wp
                        s0s = [0]
                        for t in grp[:-1]:
                            s0s.append(s0s[-1] + wts[t])
                        for t, s0 in zip(grp, s0s):
                            Wt = wts[t]
                            G = Gp[:, s0 : s0 + Wt, :]
                            # z = s[src] + d[dst(self)]
                            z = wpool.tile([P, Wt], dt.float32, tag="z")
                            sdv = G[:, 0:Wt, H // 2 : H // 2 + 1].bitcast(dt.float16)
                            nc.scalar.activation(
                                z[:],
                                sdv[:, :, 0:1].squeeze(),
                                Act.Identity,
                                bias=G[:, 0:1, H // 2 : H // 2 + 1]
                                .bitcast(dt.float16)[:, :, 1:2]
                                .rearrange("p a b -> p (a b)"),
                            )
                            # e = max(z, 0.2 z)  (leaky relu)
                            e = wpool.tile([P, Wt], dt.float32, tag="e")
                            nc.vector.scalar_tensor_tensor(
                                out=e[:], in0=z[:], scalar=NEG, in1=z[:],
                                op0=Alu.mult, op1=Alu.max,
                            )
                            # ex = exp(e), den = sum(ex)
                            ex = wpool.tile([P, Wt], dt.float16, tag="ex")
                            den = wpool.tile([P, 1], dt.float32, tag="den")
                            nc.scalar.activation(ex[:], e[:], Act.Exp, accum_out=den[:])
                            rec = wpool.tile([P, 1], dt.float32, tag="rec")
                            nc.vector.reciprocal(rec[:], den[:])
                            # num[p,f] = sum_s ex[p,s] * h16[p,s,f]
                            tmp = wpool.tile([P, H, Wt], dt.float32, tag="tmp1")
                            nc.vector.tensor_tensor(
                                out=tmp[:],
                                in0=G[:, 0:Wt, 0 : H // 2]
                                .bitcast(dt.float16)
                                .rearrange("p s f -> p f s"),
                                in1=ex[:].unsqueeze(1).to_broadcast([P, H, Wt]),
                                op=Alu.mult,
                            )
                            num = wpool.tile([P, H], dt.float32, tag="num1")
                            nc.vector.tensor_reduce(
                                num[:], tmp[:], axis=mybir.AxisListType.X, op=Alu.add
                            )
                            o1 = wpool.tile([P, H], dt.float32, tag="o1")
                            nc.vector.tensor_scalar_mul(o1[:], num[:], rec[:])
                            # h2_aug = o1 @ W2aug (+b2 pattern) on DVE
                            tmp2 = wpool.tile([P, 4, H], dt.float32, tag="tmp2")
                            nc.vector.tensor_tensor(
                                out=tmp2[:],
                                in0=o1[:].unsqueeze(1).to_broadcast([P, 4, H]),
                                in1=w2b_s[:],
                                op=Alu.mult,
                            )
                            h2t = wpool.tile([P, 4], dt.float32, tag="h2t")
                            nc.vector.tensor_reduce(
                                h2t[:], tmp2[:], axis=mybir.AxisListType.X, op=Alu.add
                            )
                            h2b = cpool.tile([P, 4], dt.float32, tag=f"h2b{t}")
                            nc.vector.tensor_add(h2b[:], h2t[:], b2a_s[:])
                            h2bs[t] = h2b
                            nc.vector.tensor_copy(rows2[:, t, :], h2b[:, 0:SL2])

                def edge_layer2(table, idx_s):
                    off = 0
                    for grp in GROUPS:
                        Wp = sum(wts[t] for t in grp)
                        ni = P * Wp
                        Gp = gpool.tile([P, Wp, ROW], dt.float32, tag="G")
                        nc.gpsimd.dma_gather(
                            out_ap=Gp[:],
                            in_ap=table[:],
                            idxs_ap=idx_s[:, off : off + 8 * Wp],
                            num_idxs=ni,
                            num_idxs_reg=ni,
                            elem_size=ROW,
                            single_packet=False,
                        )
                        off += 8 * Wp
                        s0s = [0]
                        for t in grp[:-1]:
                            s0s.append(s0s[-1] + wts[t])
                        for t, s0 in zip(grp, s0s):
                            Wt = wts[t]
                            G = Gp[:, s0 : s0 + Wt, :]
                            z = wpool.tile([P, Wt], dt.float32, tag="z")
                            nc.scalar.activation(
                                z[:],
                                G[:, 0:Wt, 1:2]
                                .bitcast(dt.float16)[:, :, 0:1]
                                .squeeze(),
                                Act.Identity,
                                bias=h2bs[t][:, 3:4],
                            )
                            e = wpool.tile([P, Wt], dt.float32, tag="e")
                            nc.vector.scalar_tensor_tensor(
                                out=e[:], in0=z[:], scalar=NEG, in1=z[:],
                                op0=Alu.mult, op1=Alu.max,
                            )
                            ex = wpool.tile([P, Wt], dt.float16, tag="ex2")
                            den = wpool.tile([P, 1], dt.float32, tag="den")
                            nc.scalar.activation(ex[:], e[:], Act.Exp, accum_out=den[:])
                            rec = wpool.tile([P, 1], dt.float32, tag="rec")
                            nc.vector.reciprocal(rec[:], den[:])
                            tmp = wpool.tile([P, C, Wt], dt.float32, tag="tmp2c")
                            nc.vector.tensor_tensor(
                                out=tmp[:],
                                in0=G[:, 0:Wt, 0:1]
                                .bitcast(dt.float16)
                                .rearrange("p s f -> p f s"),
                                in1=ex[:].unsqueeze(1).to_broadcast([P, C, Wt]),
                                op=Alu.mult,
                            )
                            num = wpool.tile([P, C], dt.float32, tag="num2")
                            nc.vector.tensor_reduce(
                                num[:], tmp[:], axis=mybir.AxisListType.X, op=Alu.add
                            )
                            nc.vector.tensor_scalar_mul(allout[:, t, :], num[:], rec[:])

                # ---- Phase C: layer-1 edge aggregation + h2
                if phase == "gemm":
                    nc.sync.dma_start(out_p[0:NH2, :], l1slabA[0:NH2, 0:C])
                    return
                if phase == "ag1":
                    nc.sync.dma_start(out_p[:], table1[0:NSH, 0:C])
                    return
                edge_layer1(table1, idx1_s)
                nc.scalar.dma_start(
                    l2slab[0:NSH, :].rearrange("(t p) c -> p t c", p=P), rows2[:]
                )
                if phase == "gat1":
                    nc.sync.dma_start(out_p[:], l2slab[0:NSH, 0:C])
                    return
                nc.gpsimd.collective_compute(
                    "AllGather",
                    Alu.bypass,
                    replica_groups=RG,
                    ins=[l2slab[:].opt()],
                    outs=[t2d[:].opt()],
                )
                nc.scalar.dma_start(
                    table2[:].bitcast(dt.float16)[0:TROWS, 0:SL2], t2d[:]
                )
                if phase == "ag2":
                    nc.sync.dma_start(out_p[:], table2[0:NSH, 0:C])  # debug only
                    return
                # ---- Phase D: layer-2 edge aggregation
                edge_layer2(table2, idx2_s)
                if phase == "gat2":
                    for t in range(NT):
                        nc.sync.dma_start(out_p[t * P : (t + 1) * P, :], allout[:, t, :])
                    return

                # ---- Phase E: global min/max + rescale
                mm = wpool.tile([P, 2], dt.float32, tag="mm")
                nc.vector.tensor_reduce(
                    mm[:, 0:1], allout[:], axis=mybir.AxisListType.XY, op=Alu.max
                )
                mnt = wpool.tile([P, 1], dt.float32, tag="mnt")
                nc.vector.tensor_reduce(
                    mnt[:], allout[:], axis=mybir.AxisListType.XY, op=Alu.min
                )
                nc.vector.tensor_scalar_mul(mm[:, 1:2], mnt[:], -1.0)
                pr = wpool.tile([P, 2], dt.float32, tag="pr")
                nc.gpsimd.partition_all_reduce(
                    pr[:], mm[:], channels=P, reduce_op=bass_rust.ReduceOp.max
                )
                nc.sync.dma_start(mmx_i[:], pr[0:1, :])
                nc.gpsimd.collective_compute(
                    "AllGather",
                    Alu.bypass,
                    replica_groups=RG,
                    ins=[mmx_i[:].opt()],
                    outs=[mmx_o[:].opt()],
                )
                mmr = wpool.tile([1, NCORES * 2], dt.float32, tag="mmr")
                nc.sync.dma_start(mmr[:], mmx_o[:])
                gmx = wpool.tile([1, 2], dt.float32, tag="gmx")
                nc.vector.tensor_reduce(
                    gmx[:],
                    mmr[:].rearrange("p (r c) -> p c r", c=2),
                    axis=mybir.AxisListType.X,
                    op=Alu.max,
                )
                bc = wpool.tile([P, 2], dt.float32, tag="bc")
                nc.gpsimd.partition_broadcast(bc[:], gmx[:])
                # scale = 2/(mx-mn); shift = 2*(-mn)/(mx-mn) - 1
                rng_ = wpool.tile([P, 1], dt.float32, tag="rng")
                nc.vector.tensor_tensor(rng_[:], bc[:, 0:1], bc[:, 1:2], op=Alu.add)
                ri = wpool.tile([P, 1], dt.float32, tag="ri")
                nc.vector.reciprocal(ri[:], rng_[:])
                sc = wpool.tile([P, 1], dt.float32, tag="sc")
                nc.vector.tensor_scalar_mul(sc[:], ri[:], 2.0)
                u = wpool.tile([P, 1], dt.float32, tag="u")
                nc.vector.tensor_tensor(u[:], bc[:, 1:2], ri[:], op=Alu.mult)
                sh = wpool.tile([P, 1], dt.float32, tag="sh")
                nc.vector.tensor_scalar(
                    out=sh[:], in0=u[:], scalar1=2.0, scalar2=-1.0,
                    op0=Alu.mult, op1=Alu.add,
                )
                fin = wpool.tile([P, NT, C], dt.float32, tag="fin")
                nc.vector.tensor_scalar(
                    out=fin[:], in0=allout[:], scalar1=sc[:], scalar2=sh[:],
                    op0=Alu.mult, op1=Alu.add,
                )
                nc.sync.dma_start(
                    out_p[:].rearrange("(t p) c -> p t c", p=P), fin[:]
                )

            _emit()
    nc.compile()
    _patch_serialization(nc)
    return nc


def _prep(x, edge_index, W1, a_src1, a_dst1, b1, W2, a_src2, a_dst2, b2):
    ei = np.asarray(edge_index).astype(np.int64)
    src_all, dst_all = ei[0], ei[1]
    counts = np.bincount(dst_all, minlength=N)
    perm_e = np.argsort(dst_all, kind="stable")
    ssorted = src_all[perm_e].astype(np.int64)
    starts = np.zeros(N + 1, np.int64)
    np.cumsum(counts, out=starts[1:])

    orders = []
    wt_core = np.zeros((NCORES, NT), np.int64)
    for c in range(NCORES):
        ids = np.arange(NSH * c, NSH * (c + 1))
        o = ids[np.argsort(-counts[ids], kind="stable")]
        orders.append(o)
        for t in range(NT):
            wt_core[c, t] = 1 + counts[o[P * t]]
    wts = tuple(int(w) for w in wt_core.max(axis=0))

    # table-1 positions (two AllGather regions: first node-halves of every
    # core, then second halves incl. per-core sentinel rows)
    NH2 = NSH // 2
    pos1 = np.empty(N + 1, np.int64)
    g = np.arange(N)
    c, o = g // NSH, g % NSH
    pos1[g] = np.where(
        o < NH2, c * NH2 + o, NCORES * NH2 + c * (NH2 + 1) + (o - NH2)
    )
    pos1[PAD] = NCORES * NH2 + NH2
    pos2 = np.empty(N + 1, np.int64)
    pos2[PAD] = PADPOS
    for c in range(NCORES):
        pos2[orders[c]] = NSHE * c + np.arange(NSH)

    idx1_maps, idx2_maps = [], []
    for c in range(NCORES):
        segs1 = []
        for t in range(NT):
            wt = wts[t]
            nodes = orders[c][P * t : P * (t + 1)]
            mat = np.full((wt, P), PAD, np.int64)
            mat[0, :] = nodes
            for p, g in enumerate(nodes):
                dg = counts[g]
                mat[1 : 1 + dg, p] = ssorted[starts[g] : starts[g] + dg]
            segs1.append(mat)
        raw = np.concatenate([m.reshape(-1) for m in segs1])
        unwrap1 = pos1[raw]
        unwrap2 = pos2[raw]

        def wrap(unwrap):
            starts_t = np.zeros(NT + 1, np.int64)
            np.cumsum([P * w for w in wts], out=starts_t[1:])
            parts = []
            for t in TILE_ORDER:
                o = starts_t[t]
                parts.append(unwrap[o : o + P * wts[t]].reshape(-1, 16).T)
            w16 = np.concatenate(parts, axis=1).astype(np.int16)
            return np.tile(w16, (NCORES, 1))

        idx1_maps.append(wrap(unwrap1))
        idx2_maps.append(wrap(unwrap2))

    bf = ml_dtypes.bfloat16
    W1aug = np.concatenate(
        [W1, (W1 @ a_src1)[:, None], (W1 @ a_dst1)[:, None]], axis=1
    ).astype(np.float32)
    w1s = (
        W1aug.reshape(KCH, P, AUG1).transpose(1, 0, 2).reshape(P, KCH * AUG1)
    ).astype(bf)
    v1 = np.concatenate([b1.astype(np.float32), np.zeros(2, np.float32)]).reshape(
        AUG1, 1
    )
    W2aug = np.concatenate(
        [W2, (W2 @ a_src2)[:, None], (W2 @ a_dst2)[:, None]], axis=1
    ).astype(np.float32)
    w2b = np.tile(W2aug.T.reshape(1, 4 * H), (P, 1)).astype(np.float32)
    b2a = np.tile(
        np.array([b2[0], b2[1], 0.0, 0.0], np.float32), (P, 1)
    ).astype(np.float32)

    x = np.asarray(x, np.float32)
    in_maps = []
    for c in range(NCORES):
        in_maps.append(
            {
                "xs": np.ascontiguousarray(x[NSH * c : NSH * (c + 1)].T).astype(bf),
                "w1s": w1s,
                "v1": v1,
                "idx1": idx1_maps[c],
                "idx2": idx2_maps[c],
                "w2b": w2b,
                "b2a": b2a,
            }
        )
    return wts, in_maps, orders


_NC_CACHE = {}


def _get_nc(wts):
    if wts not in _NC_CACHE:
        _NC_CACHE[wts] = _build(wts)
    return _NC_CACHE[wts]


def kernel(**inputs):
    from concourse.bass_utils import run_bass_kernel_spmd

    wts, in_maps, orders = _prep(
        inputs["x"], inputs["edge_index"], inputs["W1"], inputs["a_src1"],
        inputs["a_dst1"], inputs["b1"], inputs["W2"], inputs["a_src2"],
        inputs["a_dst2"], inputs["b2"],
    )
    nc = _get_nc(wts)
    res = run_bass_kernel_spmd(nc, in_maps, list(range(NCORES)))
    out = np.empty((N, C), np.float32)
    for c in range(NCORES):
        out[orders[c]] = res.results[c]["out"]
    return out
